# revision 4
# baseline (speedup 1.0000x reference)
"""ChebConv (k=2, DGL-style, lambda_max=2) on 8 Trainium2 NeuronCores.

Strategy (graph/data parallel over destination nodes), v2:
  - Host: degree/dinv, per-core edge sort into (dst-tile, src-half) chunk
    streams; layer-1 hn = x*dinv precomputed on host (bf16 tables).
  - Device, per layer: indirect-DMA gather of hn[src] rows batched 16
    chunks (2048 rows) per instruction to amortize the ~1us SWDGE fixed
    overhead; one-hot selection matrices built one DVE tensor_tensor
    (is_equal) per (tile,group) via stride-0 broadcast APs; segment-sum
    via bf16 PE matmuls; -dinv(dst) scale folded into the PSUM->SBUF
    copy; dense concat([h,x1]) @ W in bf16 with fused tanh+bias;
    PE-transpose + dinv-scale to node-major bf16 hn; ONE per-layer
    AllGather into a core-major [N,256] shared table (node ids index it;
    the two src-core-half groups read the table's row halves as slices,
    keeping dma_gather's int16 indices in range), emitted after ALL of
    the layer's gathers on the Pool queue (a collective between
    dma_gathers wedges the device).
"""

import sys

sys.path.insert(0, "/opt/trn_rl_repo")

import numpy as np
import ml_dtypes

import concourse.bacc as bacc
import concourse.bass as bass
import concourse.mybir as mybir
from concourse.bass_utils import run_bass_kernel_spmd
from concourse.masks import make_identity
from concourse.tile import TileContext
from concourse.vector_clock import ScopedClock

F32 = mybir.dt.float32
BF16 = mybir.dt.bfloat16
I32 = mybir.dt.int32
AF = mybir.ActivationFunctionType
F8 = mybir.dt.float8e4
BFNP = ml_dtypes.bfloat16

N_CORES = 8
SC = 8  # chunks per dma_gather superchunk (1024 idxs; 2048 wedges the HW)

import os

COLL_MODE = os.environ.get("KV2_COLL", "all")  # all | ag2 | none
NO_COLL = COLL_MODE == "none"
AG1_COLL = COLL_MODE == "all"
FP8 = bool(int(os.environ.get("KV2_FP8", "0")))  # fp8e4m3 hn tables
H1_CFG = int(os.environ.get("KV2_H1", "0")) or None

# ---------------------------------------------------------------------------
# walrus on this image supports only ONE sync-wait command per instruction;
# Tile freely emits several.  Split extra waits onto same-engine NoOps.
# ---------------------------------------------------------------------------


def _patched_drain_and_barrier(self, tick_clock, wait_clock):
    nc = self.nc
    probe = nc.sync.nop(nofuse=True, hint="drain_wait_split")
    wait_clock.add_sem_waits(probe.ins, ScopedClock({None: tick_clock.global_clock}))
    si = probe.ins.sync_info
    waits = list(si.on_wait) if si is not None else []
    if si is not None:
        si.on_wait = []
    for w in waits:
        self._add_instruction(
            mybir.InstNoOp(
                name=nc.get_next_instruction_name(),
                engine=mybir.EngineType.SP,
                sync_info=mybir.SyncInfo(on_wait=[w], on_update=[]),
                bass_nofuse=True,
            )
        )
    nc.sync.drain()
    nc.all_engine_barrier()
    assert self.sems is not None
    popped = nc._tile_sem_poison_stack.pop()
    assert popped is self._sem_poison
    nc.clear_and_free_semaphores(list(self.sems.allocated().values()))
    nc.all_engine_barrier()


TileContext._drain_and_barrier = _patched_drain_and_barrier


def split_sync_waits(nc):
    for f in nc.m.functions:
        for blk in f.blocks:
            insts = blk.instructions
            if not any(
                i.sync_info is not None
                and i.sync_info.on_wait
                and len(i.sync_info.on_wait) > 1
                for i in insts
            ):
                continue
            new = []
            for inst in insts:
                si = inst.sync_info
                if si is not None and si.on_wait and len(si.on_wait) > 1:
                    waits = list(si.on_wait)
                    for w in waits[:-1]:
                        new.append(
                            mybir.InstNoOp(
                                name=nc.get_next_instruction_name(),
                                engine=inst.engine,
                                sync_info=mybir.SyncInfo(on_wait=[w], on_update=[]),
                                bass_nofuse=True,
                            )
                        )
                    si.on_wait = [waits[-1]]
                new.append(inst)
            blk.instructions = new


# ---------------------------------------------------------------------------
# Host-side plan
# ---------------------------------------------------------------------------


class Plan:
    pass


def build_plan(x, src, dst, n_nodes, h1=None):
    p = Plan()
    N = n_nodes
    B = N // N_CORES          # dst nodes per core
    H1 = B // 2 if h1 is None else h1   # rows in first AllGather half
    H2 = B - H1                          # (dma_gather idx is int16: keep
    T = -(-B // 128)          # dst tiles per core   8*H < 32768)
    p.N, p.B, p.H1, p.H2, p.T = N, B, H1, H2, T
    p.last_w = B - (T - 1) * 128
    assert N // 2 < 32768  # dma_gather idx is int16

    deg = np.bincount(dst, minlength=N).astype(np.float32)
    dinv = np.where(deg > 0, 1.0 / np.sqrt(np.maximum(deg, 1.0)), 0.0).astype(
        np.float32
    )
    p.dinv = dinv

    core = dst // B
    dl = dst % B
    t = dl // 128
    d = (dl % 128).astype(np.float32)
    NH = N // 2
    g = (src >= NH).astype(np.int64)
    row = src - g * NH

    key = ((core.astype(np.int64) * T + t) * 2 + g).astype(np.int64)
    order = np.argsort(key, kind="stable")
    ks = key[order]
    cnt = np.bincount(key, minlength=N_CORES * T * 2).reshape(N_CORES, T, 2)

    # Shared (all-core) per-(tile,group) segment sizes; chunks of 128 edges
    # may SPAN tile boundaries within a group's stream (a spanning chunk is
    # multiplied against two selection matrices).
    cntmax = np.maximum(cnt.max(axis=0), 1)        # [T, 2] edges
    seg_base = np.zeros((T, 2), np.int64)
    seg_base[1:, 0] = np.cumsum(cntmax[:, 0])[:-1]
    seg_base[1:, 1] = np.cumsum(cntmax[:, 1])[:-1]
    Lg = cntmax.sum(axis=0)                        # [2]
    Cg = -(-Lg // 128)
    C = int(Cg.sum())
    p.C = C
    p.Cg = Cg.astype(np.int64)
    g_base = np.array([0, Cg[0]], np.int64)
    p.g_base = g_base

    clo = seg_base // 128                          # first chunk touching (t,g)
    chi = (seg_base + cntmax - 1) // 128           # last chunk touching (t,g)
    novl = chi - clo + 1
    dbase = np.zeros((T, 2), np.int64)
    dbase.reshape(-1)[1:] = np.cumsum(novl.reshape(-1))[:-1]
    D = int(novl.sum())
    p.D = D
    p.clo, p.chi, p.novl, p.dbase = clo, chi, novl, dbase
    p.novl_max = int(novl.max())

    # within-(core,t,g) rank of each edge
    E = len(dst)
    starts = np.zeros(N_CORES * T * 2 + 1, np.int64)
    starts[1:] = np.cumsum(cnt.reshape(-1))
    rank = np.arange(E, dtype=np.int64) - starts[ks]

    # idx16: dma_gather wrapped-16 index layout.  For group g, superchunk s
    # (SC chunks), within-superchunk flat slot j = (ch%SC)*128 + k lives at
    # partition j%16, column (g_base[g] + s*SC)*8 + j//16; the 16-row block
    # is replicated to all 128 partitions (one copy per Q7 core).
    idx16 = np.zeros((N_CORES, 16, C * 8), np.int16)
    dloc = np.full((N_CORES, 128, D), -1.0, BFNP)

    oc = core[order]
    ot = t[order]
    og = g[order]
    pos = seg_base[ot, og] + rank
    chunk = pos // 128
    part = pos % 128
    gcol = g_base[og] + chunk
    dcol = dbase[ot, og] + (chunk - clo[ot, og])
    ch_l = gcol - g_base[og]
    j = (ch_l % SC) * 128 + part
    col16 = (g_base[og] + (ch_l // SC) * SC) * 8 + j // 16
    row16 = j % 16
    idx16[oc, row16, col16] = row[order].astype(np.int16)
    p.idx16 = np.ascontiguousarray(np.tile(idx16, (1, 8, 1)))
    dloc[oc, part, dcol] = d[order].astype(BFNP)
    p.dloc = dloc

    # dinv (+/-) in node-major tile columns, per core: [cores, 128, T]
    dinvc = np.zeros((N_CORES, 128, T), np.float32)
    vb = dinv.reshape(N_CORES, B)
    for tt in range(T):
        w = min(128, B - tt * 128)
        dinvc[:, :w, tt] = vb[:, tt * 128 : tt * 128 + w]
    p.dinvc = dinvc
    p.ndinvc = (-dinvc).astype(np.float32)

    # layer-1 tables: hn1 = x * dinv, split into the two half-tables (bf16)
    hn1 = (x * dinv[:, None]).astype(np.float32)
    p.tab1a = np.ascontiguousarray(hn1[: N // 2]).astype(BFNP)
    p.tab1b = np.ascontiguousarray(hn1[N // 2 :]).astype(BFNP)

    # per-core feature-major x block (bf16)
    p.xT = np.ascontiguousarray(
        x.reshape(N_CORES, B, x.shape[1]).transpose(0, 2, 1)
    ).astype(BFNP)

    iota = np.broadcast_to(np.arange(128, dtype=np.float32), (128, 128))
    p.iota = np.ascontiguousarray(iota).astype(BFNP)
    return p


# ---------------------------------------------------------------------------
# Device program
# ---------------------------------------------------------------------------


def build_nc(p):
    B, H1, H2, T, C, D = p.B, p.H1, p.H2, p.T, p.C, p.D
    NS = -(-B // 512)  # node slabs for dense matmul
    NOVL = p.novl_max

    nc = bacc.Bacc("TRN2")
    NH = p.N // 2
    tab1a = nc.declare_dram_parameter("tab1a", [NH, 128], BF16, isOutput=False)
    tab1b = nc.declare_dram_parameter("tab1b", [NH, 128], BF16, isOutput=False)
    xT_in = nc.declare_dram_parameter("xT", [128, B], BF16, isOutput=False)
    idx_in = nc.declare_dram_parameter("idx", [128, C * 8], mybir.dt.int16, isOutput=False)
    dloc_in = nc.declare_dram_parameter("dloc", [128, D], BF16, isOutput=False)
    iota_in = nc.declare_dram_parameter("iota", [128, 128], BF16, isOutput=False)
    dinvc_in = nc.declare_dram_parameter("dinvc", [128, T], F32, isOutput=False)
    ndinvc_in = nc.declare_dram_parameter("ndinvc", [128, T], F32, isOutput=False)
    W1_in = nc.declare_dram_parameter("W1", [256, 256], BF16, isOutput=False)
    W2_in = nc.declare_dram_parameter("W2", [512, 256], BF16, isOutput=False)
    W3_in = nc.declare_dram_parameter("W3", [512, 128], BF16, isOutput=False)
    b1_in = nc.declare_dram_parameter("b1", [256, 1], F32, isOutput=False)
    b2_in = nc.declare_dram_parameter("b2", [256, 1], F32, isOutput=False)
    b3_in = nc.declare_dram_parameter("b3", [128, 1], F32, isOutput=False)
    out_p = nc.declare_dram_parameter("outT", [128, B], F32, isOutput=True)

    TD = F8 if FP8 else BF16
    hn_stage = nc.dram_tensor("hn_stage", [B, 256], TD)
    hn_sh = nc.dram_tensor("hn_sh", [p.N, 256], TD, addr_space="Shared")

    clo, chi, novl, dbase = p.clo, p.chi, p.novl, p.dbase
    Cg, g_base = p.Cg, p.g_base

    from contextlib import ExitStack

    with TileContext(nc) as tc, ExitStack() as es:
        cst = es.enter_context(tc.tile_pool(name="cst", bufs=1))
        gp1 = es.enter_context(tc.tile_pool(name="gp1", bufs=3))
        gp2 = es.enter_context(tc.tile_pool(name="gp2", bufs=3))
        mp = es.enter_context(tc.tile_pool(name="mp", bufs=3))
        evp = es.enter_context(tc.tile_pool(name="evp", bufs=4))
        hnp = es.enter_context(tc.tile_pool(name="hnp", bufs=4))
        agg_ps = es.enter_context(tc.tile_pool(name="agg_ps", bufs=4, space="PSUM"))
        tr_ps = es.enter_context(tc.tile_pool(name="tr_ps", bufs=2, space="PSUM"))
        dn_ps = es.enter_context(tc.tile_pool(name="dn_ps", bufs=2, space="PSUM"))

        # ---- constants ----
        idx_t = cst.tile([128, C * 8], mybir.dt.int16, tag="idx")
        dloc_t = cst.tile([128, D], BF16, tag="dloc")
        iota_t = cst.tile([128, 128], BF16, tag="iota")
        dinvc_t = cst.tile([128, T], F32, tag="dinvc")
        ndinvc_t = cst.tile([128, T], F32, tag="ndinvc")
        ident = cst.tile([128, 128], BF16, tag="ident")
        nc.sync.dma_start(out=idx_t[:], in_=idx_in[:])
        nc.sync.dma_start(out=dloc_t[:], in_=dloc_in[:])
        nc.sync.dma_start(out=iota_t[:], in_=iota_in[:])
        nc.sync.dma_start(out=dinvc_t[:], in_=dinvc_in[:])
        nc.sync.dma_start(out=ndinvc_t[:], in_=ndinvc_in[:])
        make_identity(nc, ident[:])

        def load_w(w_in, K, FO):
            tiles = []
            for kk in range(K // 128):
                row = []
                for fo in range(FO // 128):
                    wt = cst.tile(
                        [128, 128], BF16,
                        tag=f"w{w_in.name}_{kk}_{fo}", name=f"w{w_in.name}_{kk}_{fo}",
                    )
                    nc.sync.dma_start(
                        out=wt[:],
                        in_=w_in[kk * 128 : (kk + 1) * 128, fo * 128 : (fo + 1) * 128],
                    )
                    row.append(wt)
                tiles.append(row)
            return tiles

        W1t = load_w(W1_in, 256, 256)
        W2t = load_w(W2_in, 512, 256)
        W3t = load_w(W3_in, 512, 128)
        bt = {}
        for name, b_in, FO in (("b1", b1_in, 256), ("b2", b2_in, 256), ("b3", b3_in, 128)):
            bt[name] = []
            for fo in range(FO // 128):
                btile = cst.tile([128, 1], F32, tag=f"{name}_{fo}", name=f"{name}_{fo}")
                nc.sync.dma_start(out=btile[:], in_=b_in[fo * 128 : (fo + 1) * 128, :])
                bt[name].append(btile)

        # ---- persistent activations (feature-major, bf16) ----
        hA = [cst.tile([128, B], BF16, tag=f"hA{i}", name=f"hA{i}") for i in range(2)]
        hB = [cst.tile([128, B], BF16, tag=f"hB{i}", name=f"hB{i}") for i in range(2)]
        x1 = [cst.tile([128, B], BF16, tag=f"x1_{i}", name=f"x1_{i}") for i in range(2)]
        outF = cst.tile([128, B], F32, tag="outF", name="outF")
        nc.sync.dma_start(out=hA[0][:], in_=xT_in[:])

        layers = [
            (128, 256, W1t, bt["b1"], AF.Tanh),
            (256, 256, W2t, bt["b2"], AF.Tanh),
            (256, 256, W2t, bt["b2"], AF.Tanh),
            (256, 256, W2t, bt["b2"], AF.Tanh),
            (256, 128, W3t, bt["b3"], AF.Identity),
        ]

        cur, nxt = hA, hB
        for li, (FI, FO, Wt, bias, act) in enumerate(layers):
            if li == 0:
                tables = (tab1a[:], tab1b[:])
            else:
                tables = (hn_sh[0:NH, :], hn_sh[NH : p.N, :])
            nh = FI // 128
            last = li == len(layers) - 1
            gpool = gp1 if li == 0 else gp2
            GD = BF16 if li == 0 else TD

            issued = [{}, {}]  # per-group: superchunk -> G tile

            def need_superchunks(gg, up_to_chunk):
                s_hi = up_to_chunk // SC
                for s in range(len(issued[gg]), s_hi + 1):
                    scw = min(SC, int(Cg[gg]) - s * SC)
                    G = gpool.tile([128, SC * FI], GD, tag=f"g{gg}")
                    cb = (int(g_base[gg]) + s * SC) * 8
                    nc.gpsimd.dma_gather(
                        out_ap=G[:, : scw * FI].rearrange("p (a b) -> p a b", a=scw),
                        in_ap=tables[gg],
                        idxs_ap=idx_t[:, cb : cb + scw * 8],
                        num_idxs=scw * 128,
                        num_idxs_reg=scw * 128,
                        elem_size=FI,
                    )
                    issued[gg][s] = G

            def agg_tile(gg, tt):
                """Gather+segment-sum for (group gg, tile tt) -> PSUM agg."""
                lo, hi = int(clo[tt, gg]), int(chi[tt, gg])
                need_superchunks(gg, hi)
                nv = hi - lo + 1
                db = int(dbase[tt, gg])
                M = mp.tile([128, NOVL * 128], GD, tag="m")
                nc.vector.tensor_tensor(
                    out=M[:, : nv * 128].rearrange("p (a b) -> p a b", a=nv),
                    in0=iota_t[:].unsqueeze(1).broadcast_to([128, nv, 128]),
                    in1=dloc_t[:, db : db + nv].unsqueeze(2).broadcast_to(
                        [128, nv, 128]
                    ),
                    op=mybir.AluOpType.is_equal,
                )
                agg = agg_ps.tile([128, 256], F32, tag="agg", space="PSUM")
                for k, ch in enumerate(range(lo, hi + 1)):
                    s, c = divmod(ch, SC)
                    G = issued[gg][s]
                    nc.tensor.matmul(
                        out=agg[:, :FI],
                        lhsT=M[:, k * 128 : (k + 1) * 128],
                        rhs=G[:, c * FI : (c + 1) * FI],
                        start=(k == 0),
                        stop=(k == nv - 1),
                    )
                return agg

            def x1_from_agg(gg, tt, agg):
                """Scale by -dinv(dst), transpose to feature-major, into x1."""
                tw = min(128, B - tt * 128)
                x1nm = evp.tile([128, 256], BF16, tag="x1nm")
                nc.scalar.activation(
                    out=x1nm[:, :FI],
                    in_=agg[:, :FI],
                    func=AF.Copy,
                    scale=ndinvc_t[:, tt : tt + 1],
                )
                for hh in range(nh):
                    trp = tr_ps.tile([128, 128], BF16, tag="tr", space="PSUM")
                    nc.tensor.transpose(
                        out=trp[:],
                        in_=x1nm[:, hh * 128 : (hh + 1) * 128],
                        identity=ident[:],
                    )
                    if gg == 0:
                        nc.vector.tensor_copy(
                            out=x1[hh][:, tt * 128 : tt * 128 + tw],
                            in_=trp[:, :tw],
                        )
                    else:
                        nc.vector.tensor_add(
                            out=x1[hh][:, tt * 128 : tt * 128 + tw],
                            in0=x1[hh][:, tt * 128 : tt * 128 + tw],
                            in1=trp[:, :tw],
                        )

            def agg_tile_merged(tt):
                """Both groups' chunks into one PSUM accumulation (layer 1:
                tables are host-provided, no collective dependency, so the
                two-pass split buys nothing)."""
                agg = agg_ps.tile([128, 256], F32, tag="agg", space="PSUM")
                spans = []
                for gg in range(2):
                    lo, hi = int(clo[tt, gg]), int(chi[tt, gg])
                    need_superchunks(gg, hi)
                    spans.append((gg, lo, hi, int(dbase[tt, gg])))
                first = True
                for gg, lo, hi, db in spans:
                    nv = hi - lo + 1
                    M = mp.tile([128, NOVL * 128], GD, tag="m")
                    nc.vector.tensor_tensor(
                        out=M[:, : nv * 128].rearrange("p (a b) -> p a b", a=nv),
                        in0=iota_t[:].unsqueeze(1).broadcast_to([128, nv, 128]),
                        in1=dloc_t[:, db : db + nv].unsqueeze(2).broadcast_to(
                            [128, nv, 128]
                        ),
                        op=mybir.AluOpType.is_equal,
                    )
                    for k, ch in enumerate(range(lo, hi + 1)):
                        s, c = divmod(ch, SC)
                        G = issued[gg][s]
                        nc.tensor.matmul(
                            out=agg[:, :FI],
                            lhsT=M[:, k * 128 : (k + 1) * 128],
                            rhs=G[:, c * FI : (c + 1) * FI],
                            start=first,
                            stop=(gg == 1 and k == nv - 1),
                        )
                        first = False
                return agg

            # -------- pass A: group 0 over all tiles (layers 2+) --------
            if li > 0:
                for tt in range(T):
                    x1_from_agg(0, tt, agg_tile(0, tt))

            # -------- pass B: group 1, slab-interleaved dense + hn --------
            rhs_list = [cur[i] for i in range(nh)] + [x1[i] for i in range(nh)]
            for s in range(NS):
                s0 = s * 512
                sw = min(512, B - s0)
                for tt in range(4 * s, min(4 * s + 4, T)):
                    if li == 0:
                        x1_from_agg(0, tt, agg_tile_merged(tt))
                    else:
                        x1_from_agg(1, tt, agg_tile(1, tt))
                for fo in range(FO // 128):
                    dps = dn_ps.tile([128, 512], F32, tag="dn", space="PSUM")
                    for kk in range(2 * nh):
                        nc.tensor.matmul(
                            out=dps[:, :sw],
                            lhsT=Wt[kk][fo][:],
                            rhs=rhs_list[kk][:, s0 : s0 + sw],
                            start=(kk == 0),
                            stop=(kk == 2 * nh - 1),
                        )
                    nc.scalar.activation(
                        out=(outF if last else nxt[fo])[:, s0 : s0 + sw],
                        in_=dps[:, :sw],
                        func=act,
                        bias=bias[fo][:],
                    )
                if not last:
                    for tt in range(4 * s, min(4 * s + 4, T)):
                        tw = min(128, B - tt * 128)
                        hn_nm = hnp.tile([128, 256], TD, tag="hn_nm")
                        for hh in range(FO // 128):
                            trp = tr_ps.tile([128, 128], BF16, tag="tr", space="PSUM")
                            nc.tensor.transpose(
                                out=trp[:tw, :],
                                in_=nxt[hh][:, tt * 128 : tt * 128 + tw],
                                identity=ident[:],
                            )
                            nc.scalar.activation(
                                out=hn_nm[:tw, hh * 128 : (hh + 1) * 128],
                                in_=trp[:tw, :],
                                func=AF.Copy,
                                scale=dinvc_t[:tw, tt : tt + 1],
                            )
                        nc.scalar.dma_start(
                            out=hn_stage[tt * 128 : tt * 128 + tw, :], in_=hn_nm[:tw, :]
                        )
            if not last:
                # ONE AllGather per layer (output is core-major, so node ids
                # index the table; groups read the two row-halves as slices).
                # Emitted AFTER all of this layer's gathers on the Pool queue
                # (a collective between dma_gathers wedges the device).
                nc.gpsimd.collective_compute(
                    "AllGather",
                    mybir.AluOpType.bypass,
                    replica_groups=[list(range(N_CORES))],
                    ins=[hn_stage[0:B, :]],
                    outs=[hn_sh[:]],
                )
            cur, nxt = nxt, cur

        nc.sync.dma_start(out=out_p[:], in_=outF[:])

    nc.compile()
    split_sync_waits(nc)
    bass.Bass.finalize(nc)
    return nc


# ---------------------------------------------------------------------------
# Entry point
# ---------------------------------------------------------------------------


def make_in_maps(p, W1, b1, W2, b2, W3, b3):
    in_maps = []
    for c in range(N_CORES):
        in_maps.append(
            {
                "tab1a": p.tab1a,
                "tab1b": p.tab1b,
                "xT": p.xT[c],
                "idx": p.idx16[c],
                "dloc": p.dloc[c],
                "iota": p.iota,
                "dinvc": p.dinvc[c],
                "ndinvc": p.ndinvc[c],
                "W1": np.asarray(W1, np.float32).astype(BFNP),
                "W2": np.asarray(W2, np.float32).astype(BFNP),
                "W3": np.asarray(W3, np.float32).astype(BFNP),
                "b1": np.asarray(b1, np.float32).reshape(-1, 1),
                "b2": np.asarray(b2, np.float32).reshape(-1, 1),
                "b3": np.asarray(b3, np.float32).reshape(-1, 1),
            }
        )
    return in_maps


def kernel(x, src, dst, W1, b1, W2, b2, W3, b3):
    x = np.asarray(x, np.float32)
    src = np.asarray(src, np.int32)
    dst = np.asarray(dst, np.int32)
    p = build_plan(x, src, dst, x.shape[0], h1=H1_CFG)
    nc = build_nc(p)
    in_maps = make_in_maps(p, W1, b1, W2, b2, W3, b3)
    res = run_bass_kernel_spmd(nc, in_maps, list(range(N_CORES))).results
    out = np.empty((x.shape[0], W3.shape[1]), np.float32)
    B = p.B
    for c in range(N_CORES):
        out[c * B : (c + 1) * B, :] = res[c]["outT"].T
    return out


# revision 7
# speedup vs baseline: 1.0653x; 1.0653x over previous
"""ChebConv (k=2, DGL-style, lambda_max=2) on 8 Trainium2 NeuronCores.

Strategy (graph/data parallel over destination nodes), v2:
  - Host: degree/dinv, per-core edge sort into (dst-tile, src-half) chunk
    streams; layer-1 hn = x*dinv precomputed on host (bf16 tables).
  - Device, per layer: indirect-DMA gather of hn[src] rows batched 16
    chunks (2048 rows) per instruction to amortize the ~1us SWDGE fixed
    overhead; one-hot selection matrices built one DVE tensor_tensor
    (is_equal) per (tile,group) via stride-0 broadcast APs; segment-sum
    via bf16 PE matmuls; -dinv(dst) scale folded into the PSUM->SBUF
    copy; dense concat([h,x1]) @ W in bf16 with fused tanh+bias;
    PE-transpose + dinv-scale to node-major bf16 hn; per-layer
    AllGather in two src-halves (first fires mid-dense) so the next
    layer's first-half gathers overlap the second collective.
"""

import sys

sys.path.insert(0, "/opt/trn_rl_repo")

import numpy as np
import ml_dtypes

import concourse.bacc as bacc
import concourse.bass as bass
import concourse.mybir as mybir
from concourse.bass_utils import run_bass_kernel_spmd
from concourse.masks import make_identity
from concourse.tile import TileContext
from concourse.vector_clock import ScopedClock

F32 = mybir.dt.float32
BF16 = mybir.dt.bfloat16
I32 = mybir.dt.int32
AF = mybir.ActivationFunctionType
F8 = mybir.dt.float8e4
BFNP = ml_dtypes.bfloat16

N_CORES = 8
SC = 8  # chunks per dma_gather superchunk (1024 idxs; 2048 wedges the HW)

import os

COLL_MODE = os.environ.get("KV2_COLL", "all")  # all | ag2 | none
NO_COLL = COLL_MODE == "none"
AG1_COLL = COLL_MODE == "all"
FP8 = bool(int(os.environ.get("KV2_FP8", "0")))  # fp8e4m3 hn tables
H1_CFG = int(os.environ.get("KV2_H1", "0")) or None

# ---------------------------------------------------------------------------
# walrus on this image supports only ONE sync-wait command per instruction;
# Tile freely emits several.  Split extra waits onto same-engine NoOps.
# ---------------------------------------------------------------------------


def _patched_drain_and_barrier(self, tick_clock, wait_clock):
    nc = self.nc
    probe = nc.sync.nop(nofuse=True, hint="drain_wait_split")
    wait_clock.add_sem_waits(probe.ins, ScopedClock({None: tick_clock.global_clock}))
    si = probe.ins.sync_info
    waits = list(si.on_wait) if si is not None else []
    if si is not None:
        si.on_wait = []
    for w in waits:
        self._add_instruction(
            mybir.InstNoOp(
                name=nc.get_next_instruction_name(),
                engine=mybir.EngineType.SP,
                sync_info=mybir.SyncInfo(on_wait=[w], on_update=[]),
                bass_nofuse=True,
            )
        )
    nc.sync.drain()
    nc.all_engine_barrier()
    assert self.sems is not None
    popped = nc._tile_sem_poison_stack.pop()
    assert popped is self._sem_poison
    nc.clear_and_free_semaphores(list(self.sems.allocated().values()))
    nc.all_engine_barrier()


TileContext._drain_and_barrier = _patched_drain_and_barrier


def split_sync_waits(nc):
    for f in nc.m.functions:
        for blk in f.blocks:
            insts = blk.instructions
            if not any(
                i.sync_info is not None
                and i.sync_info.on_wait
                and len(i.sync_info.on_wait) > 1
                for i in insts
            ):
                continue
            new = []
            for inst in insts:
                si = inst.sync_info
                if si is not None and si.on_wait and len(si.on_wait) > 1:
                    waits = list(si.on_wait)
                    for w in waits[:-1]:
                        new.append(
                            mybir.InstNoOp(
                                name=nc.get_next_instruction_name(),
                                engine=inst.engine,
                                sync_info=mybir.SyncInfo(on_wait=[w], on_update=[]),
                                bass_nofuse=True,
                            )
                        )
                    si.on_wait = [waits[-1]]
                new.append(inst)
            blk.instructions = new


# ---------------------------------------------------------------------------
# Host-side plan
# ---------------------------------------------------------------------------


class Plan:
    pass


def build_plan(x, src, dst, n_nodes, h1=None):
    p = Plan()
    N = n_nodes
    B = N // N_CORES          # dst nodes per core
    H1 = B // 2 if h1 is None else h1   # rows in first AllGather half
    H2 = B - H1                          # (dma_gather idx is int16: keep
    T = -(-B // 128)          # dst tiles per core   8*H < 32768)
    p.N, p.B, p.H1, p.H2, p.T = N, B, H1, H2, T
    p.last_w = B - (T - 1) * 128
    assert N // 2 < 32768  # dma_gather idx is int16

    deg = np.bincount(dst, minlength=N).astype(np.float32)
    dinv = np.where(deg > 0, 1.0 / np.sqrt(np.maximum(deg, 1.0)), 0.0).astype(
        np.float32
    )
    p.dinv = dinv

    core = dst // B
    dl = dst % B
    t = dl // 128
    d = (dl % 128).astype(np.float32)
    NH = N // 2
    g = (src >= NH).astype(np.int64)
    row = src - g * NH

    key = ((core.astype(np.int64) * T + t) * 2 + g).astype(np.int64)
    order = np.argsort(key, kind="stable")
    ks = key[order]
    cnt = np.bincount(key, minlength=N_CORES * T * 2).reshape(N_CORES, T, 2)

    # Shared (all-core) per-(tile,group) segment sizes; chunks of 128 edges
    # may SPAN tile boundaries within a group's stream (a spanning chunk is
    # multiplied against two selection matrices).
    cntmax = np.maximum(cnt.max(axis=0), 1)        # [T, 2] edges
    seg_base = np.zeros((T, 2), np.int64)
    seg_base[1:, 0] = np.cumsum(cntmax[:, 0])[:-1]
    seg_base[1:, 1] = np.cumsum(cntmax[:, 1])[:-1]
    Lg = cntmax.sum(axis=0)                        # [2]
    Cg = -(-Lg // 128)
    C = int(Cg.sum())
    p.C = C
    p.Cg = Cg.astype(np.int64)
    g_base = np.array([0, Cg[0]], np.int64)
    p.g_base = g_base

    clo = seg_base // 128                          # first chunk touching (t,g)
    chi = (seg_base + cntmax - 1) // 128           # last chunk touching (t,g)
    novl = chi - clo + 1
    dbase = np.zeros((T, 2), np.int64)
    dbase.reshape(-1)[1:] = np.cumsum(novl.reshape(-1))[:-1]
    D = int(novl.sum())
    p.D = D
    p.clo, p.chi, p.novl, p.dbase = clo, chi, novl, dbase
    p.novl_max = int(novl.max())

    # within-(core,t,g) rank of each edge
    E = len(dst)
    starts = np.zeros(N_CORES * T * 2 + 1, np.int64)
    starts[1:] = np.cumsum(cnt.reshape(-1))
    rank = np.arange(E, dtype=np.int64) - starts[ks]

    # idx16: dma_gather wrapped-16 index layout.  For group g, superchunk s
    # (SC chunks), within-superchunk flat slot j = (ch%SC)*128 + k lives at
    # partition j%16, column (g_base[g] + s*SC)*8 + j//16; the 16-row block
    # is replicated to all 128 partitions (one copy per Q7 core).
    idx16 = np.zeros((N_CORES, 16, C * 8), np.int16)
    dloc = np.full((N_CORES, 128, D), -1.0, BFNP)

    oc = core[order]
    ot = t[order]
    og = g[order]
    pos = seg_base[ot, og] + rank
    chunk = pos // 128
    part = pos % 128
    gcol = g_base[og] + chunk
    dcol = dbase[ot, og] + (chunk - clo[ot, og])
    ch_l = gcol - g_base[og]
    j = (ch_l % SC) * 128 + part
    col16 = (g_base[og] + (ch_l // SC) * SC) * 8 + j // 16
    row16 = j % 16
    idx16[oc, row16, col16] = row[order].astype(np.int16)
    p.idx16 = np.ascontiguousarray(np.tile(idx16, (1, 8, 1)))
    dloc[oc, part, dcol] = d[order].astype(BFNP)
    p.dloc = dloc

    # dinv (+/-) in node-major tile columns, per core: [cores, 128, T]
    dinvc = np.zeros((N_CORES, 128, T), np.float32)
    vb = dinv.reshape(N_CORES, B)
    for tt in range(T):
        w = min(128, B - tt * 128)
        dinvc[:, :w, tt] = vb[:, tt * 128 : tt * 128 + w]
    p.dinvc = dinvc
    p.ndinvc = (-dinvc).astype(np.float32)

    # layer-1 tables: hn1 = x * dinv, split into the two half-tables (bf16)
    hn1 = (x * dinv[:, None]).astype(np.float32)
    p.tab1a = np.ascontiguousarray(hn1[: N // 2]).astype(BFNP)
    p.tab1b = np.ascontiguousarray(hn1[N // 2 :]).astype(BFNP)

    # per-core feature-major x block (bf16)
    p.xT = np.ascontiguousarray(
        x.reshape(N_CORES, B, x.shape[1]).transpose(0, 2, 1)
    ).astype(BFNP)

    iota = np.broadcast_to(np.arange(128, dtype=np.float32), (128, 128))
    p.iota = np.ascontiguousarray(iota).astype(BFNP)
    return p


# ---------------------------------------------------------------------------
# Device program
# ---------------------------------------------------------------------------


def build_nc(p):
    B, H1, H2, T, C, D = p.B, p.H1, p.H2, p.T, p.C, p.D
    NS = -(-B // 512)  # node slabs for dense matmul
    NOVL = p.novl_max

    nc = bacc.Bacc("TRN2")
    NH = p.N // 2
    tab1a = nc.declare_dram_parameter("tab1a", [NH, 128], BF16, isOutput=False)
    tab1b = nc.declare_dram_parameter("tab1b", [NH, 128], BF16, isOutput=False)
    xT_in = nc.declare_dram_parameter("xT", [128, B], BF16, isOutput=False)
    idx_in = nc.declare_dram_parameter("idx", [128, C * 8], mybir.dt.int16, isOutput=False)
    dloc_in = nc.declare_dram_parameter("dloc", [128, D], BF16, isOutput=False)
    iota_in = nc.declare_dram_parameter("iota", [128, 128], BF16, isOutput=False)
    dinvc_in = nc.declare_dram_parameter("dinvc", [128, T], F32, isOutput=False)
    ndinvc_in = nc.declare_dram_parameter("ndinvc", [128, T], F32, isOutput=False)
    W1_in = nc.declare_dram_parameter("W1", [256, 256], BF16, isOutput=False)
    W2_in = nc.declare_dram_parameter("W2", [512, 256], BF16, isOutput=False)
    W3_in = nc.declare_dram_parameter("W3", [512, 128], BF16, isOutput=False)
    b1_in = nc.declare_dram_parameter("b1", [256, 1], F32, isOutput=False)
    b2_in = nc.declare_dram_parameter("b2", [256, 1], F32, isOutput=False)
    b3_in = nc.declare_dram_parameter("b3", [128, 1], F32, isOutput=False)
    out_p = nc.declare_dram_parameter("outT", [128, B], F32, isOutput=True)

    TD = F8 if FP8 else BF16
    # ping-pong staging: layer i stages into buffer i%2 so the next layer's
    # staging writes can never race the in-flight AllGather's read
    hn_stage = [nc.dram_tensor(f"hn_stage{i}", [B, 256], TD) for i in range(2)]
    hn_sh = nc.dram_tensor("hn_sh", [p.N, 256], TD, addr_space="Shared")

    clo, chi, novl, dbase = p.clo, p.chi, p.novl, p.dbase
    Cg, g_base = p.Cg, p.g_base

    from contextlib import ExitStack

    with TileContext(nc) as tc, ExitStack() as es:
        cst = es.enter_context(tc.tile_pool(name="cst", bufs=1))
        gp1 = es.enter_context(tc.tile_pool(name="gp1", bufs=3))
        gp2 = es.enter_context(tc.tile_pool(name="gp2", bufs=3))
        mp = es.enter_context(tc.tile_pool(name="mp", bufs=3))
        evp = es.enter_context(tc.tile_pool(name="evp", bufs=4))
        hnp = es.enter_context(tc.tile_pool(name="hnp", bufs=4))
        agg_ps = es.enter_context(tc.tile_pool(name="agg_ps", bufs=4, space="PSUM"))
        tr_ps = es.enter_context(tc.tile_pool(name="tr_ps", bufs=2, space="PSUM"))
        dn_ps = es.enter_context(tc.tile_pool(name="dn_ps", bufs=2, space="PSUM"))

        # ---- constants ----
        idx_t = cst.tile([128, C * 8], mybir.dt.int16, tag="idx")
        dloc_t = cst.tile([128, D], BF16, tag="dloc")
        iota_t = cst.tile([128, 128], BF16, tag="iota")
        dinvc_t = cst.tile([128, T], F32, tag="dinvc")
        ndinvc_t = cst.tile([128, T], F32, tag="ndinvc")
        ident = cst.tile([128, 128], BF16, tag="ident")
        nc.sync.dma_start(out=idx_t[:], in_=idx_in[:])
        nc.sync.dma_start(out=dloc_t[:], in_=dloc_in[:])
        nc.sync.dma_start(out=iota_t[:], in_=iota_in[:])
        nc.sync.dma_start(out=dinvc_t[:], in_=dinvc_in[:])
        nc.sync.dma_start(out=ndinvc_t[:], in_=ndinvc_in[:])
        make_identity(nc, ident[:])

        def load_w(w_in, K, FO):
            tiles = []
            for kk in range(K // 128):
                row = []
                for fo in range(FO // 128):
                    wt = cst.tile(
                        [128, 128], BF16,
                        tag=f"w{w_in.name}_{kk}_{fo}", name=f"w{w_in.name}_{kk}_{fo}",
                    )
                    nc.sync.dma_start(
                        out=wt[:],
                        in_=w_in[kk * 128 : (kk + 1) * 128, fo * 128 : (fo + 1) * 128],
                    )
                    row.append(wt)
                tiles.append(row)
            return tiles

        W1t = load_w(W1_in, 256, 256)
        W2t = load_w(W2_in, 512, 256)
        W3t = load_w(W3_in, 512, 128)
        bt = {}
        for name, b_in, FO in (("b1", b1_in, 256), ("b2", b2_in, 256), ("b3", b3_in, 128)):
            bt[name] = []
            for fo in range(FO // 128):
                btile = cst.tile([128, 1], F32, tag=f"{name}_{fo}", name=f"{name}_{fo}")
                nc.sync.dma_start(out=btile[:], in_=b_in[fo * 128 : (fo + 1) * 128, :])
                bt[name].append(btile)

        # ---- persistent activations (feature-major, bf16) ----
        hA = [cst.tile([128, B], BF16, tag=f"hA{i}", name=f"hA{i}") for i in range(2)]
        hB = [cst.tile([128, B], BF16, tag=f"hB{i}", name=f"hB{i}") for i in range(2)]
        x1 = [cst.tile([128, B], BF16, tag=f"x1_{i}", name=f"x1_{i}") for i in range(2)]
        outF = cst.tile([128, B], F32, tag="outF", name="outF")
        nc.sync.dma_start(out=hA[0][:], in_=xT_in[:])

        layers = [
            (128, 256, W1t, bt["b1"], AF.Tanh),
            (256, 256, W2t, bt["b2"], AF.Tanh),
            (256, 256, W2t, bt["b2"], AF.Tanh),
            (256, 256, W2t, bt["b2"], AF.Tanh),
            (256, 128, W3t, bt["b3"], AF.Identity),
        ]

        cur, nxt = hA, hB
        for li, (FI, FO, Wt, bias, act) in enumerate(layers):
            if li == 0:
                tables = (tab1a[:], tab1b[:])
            else:
                tables = (hn_sh[0:NH, :], hn_sh[NH : p.N, :])
            nh = FI // 128
            last = li == len(layers) - 1
            gpool = gp1 if li == 0 else gp2
            GD = BF16 if li == 0 else TD

            issued = [{}, {}]  # per-group: superchunk -> G tile

            def need_superchunks(gg, up_to_chunk):
                s_hi = up_to_chunk // SC
                for s in range(len(issued[gg]), s_hi + 1):
                    scw = min(SC, int(Cg[gg]) - s * SC)
                    G = gpool.tile([128, SC * FI], GD, tag=f"g{gg}")
                    cb = (int(g_base[gg]) + s * SC) * 8
                    nc.gpsimd.dma_gather(
                        out_ap=G[:, : scw * FI].rearrange("p (a b) -> p a b", a=scw),
                        in_ap=tables[gg],
                        idxs_ap=idx_t[:, cb : cb + scw * 8],
                        num_idxs=scw * 128,
                        num_idxs_reg=scw * 128,
                        elem_size=FI,
                    )
                    issued[gg][s] = G

            def agg_tile(gg, tt):
                """Gather+segment-sum for (group gg, tile tt) -> PSUM agg."""
                lo, hi = int(clo[tt, gg]), int(chi[tt, gg])
                need_superchunks(gg, hi)
                nv = hi - lo + 1
                db = int(dbase[tt, gg])
                M = mp.tile([128, NOVL * 128], GD, tag="m")
                nc.vector.tensor_tensor(
                    out=M[:, : nv * 128].rearrange("p (a b) -> p a b", a=nv),
                    in0=iota_t[:].unsqueeze(1).broadcast_to([128, nv, 128]),
                    in1=dloc_t[:, db : db + nv].unsqueeze(2).broadcast_to(
                        [128, nv, 128]
                    ),
                    op=mybir.AluOpType.is_equal,
                )
                agg = agg_ps.tile([128, 256], F32, tag="agg", space="PSUM")
                for k, ch in enumerate(range(lo, hi + 1)):
                    s, c = divmod(ch, SC)
                    G = issued[gg][s]
                    nc.tensor.matmul(
                        out=agg[:, :FI],
                        lhsT=M[:, k * 128 : (k + 1) * 128],
                        rhs=G[:, c * FI : (c + 1) * FI],
                        start=(k == 0),
                        stop=(k == nv - 1),
                    )
                return agg

            def x1_from_agg(gg, tt, agg):
                """Scale by -dinv(dst), transpose to feature-major, into x1."""
                tw = min(128, B - tt * 128)
                x1nm = evp.tile([128, 256], BF16, tag="x1nm")
                nc.scalar.activation(
                    out=x1nm[:, :FI],
                    in_=agg[:, :FI],
                    func=AF.Copy,
                    scale=ndinvc_t[:, tt : tt + 1],
                )
                for hh in range(nh):
                    trp = tr_ps.tile([128, 128], BF16, tag="tr", space="PSUM")
                    nc.tensor.transpose(
                        out=trp[:],
                        in_=x1nm[:, hh * 128 : (hh + 1) * 128],
                        identity=ident[:],
                    )
                    if gg == 0:
                        nc.vector.tensor_copy(
                            out=x1[hh][:, tt * 128 : tt * 128 + tw],
                            in_=trp[:, :tw],
                        )
                    else:
                        nc.vector.tensor_add(
                            out=x1[hh][:, tt * 128 : tt * 128 + tw],
                            in0=x1[hh][:, tt * 128 : tt * 128 + tw],
                            in1=trp[:, :tw],
                        )

            def agg_tile_merged(tt):
                """Both groups' chunks into one PSUM accumulation (layer 1:
                tables are host-provided, no collective dependency, so the
                two-pass split buys nothing)."""
                agg = agg_ps.tile([128, 256], F32, tag="agg", space="PSUM")
                spans = []
                for gg in range(2):
                    lo, hi = int(clo[tt, gg]), int(chi[tt, gg])
                    need_superchunks(gg, hi)
                    spans.append((gg, lo, hi, int(dbase[tt, gg])))
                first = True
                for gg, lo, hi, db in spans:
                    nv = hi - lo + 1
                    M = mp.tile([128, NOVL * 128], GD, tag="m")
                    nc.vector.tensor_tensor(
                        out=M[:, : nv * 128].rearrange("p (a b) -> p a b", a=nv),
                        in0=iota_t[:].unsqueeze(1).broadcast_to([128, nv, 128]),
                        in1=dloc_t[:, db : db + nv].unsqueeze(2).broadcast_to(
                            [128, nv, 128]
                        ),
                        op=mybir.AluOpType.is_equal,
                    )
                    for k, ch in enumerate(range(lo, hi + 1)):
                        s, c = divmod(ch, SC)
                        G = issued[gg][s]
                        nc.tensor.matmul(
                            out=agg[:, :FI],
                            lhsT=M[:, k * 128 : (k + 1) * 128],
                            rhs=G[:, c * FI : (c + 1) * FI],
                            start=first,
                            stop=(gg == 1 and k == nv - 1),
                        )
                        first = False
                return agg

            # -------- pass A: group 0 over all tiles (layers 2+) --------
            if li > 0:
                for tt in range(T):
                    x1_from_agg(0, tt, agg_tile(0, tt))

            # -------- pass B: group 1, slab-interleaved dense + hn --------
            rhs_list = [cur[i] for i in range(nh)] + [x1[i] for i in range(nh)]
            for s in range(NS):
                s0 = s * 512
                sw = min(512, B - s0)
                for tt in range(4 * s, min(4 * s + 4, T)):
                    if li == 0:
                        x1_from_agg(0, tt, agg_tile_merged(tt))
                    else:
                        x1_from_agg(1, tt, agg_tile(1, tt))
                for fo in range(FO // 128):
                    dps = dn_ps.tile([128, 512], F32, tag="dn", space="PSUM")
                    for kk in range(2 * nh):
                        nc.tensor.matmul(
                            out=dps[:, :sw],
                            lhsT=Wt[kk][fo][:],
                            rhs=rhs_list[kk][:, s0 : s0 + sw],
                            start=(kk == 0),
                            stop=(kk == 2 * nh - 1),
                        )
                    nc.scalar.activation(
                        out=(outF if last else nxt[fo])[:, s0 : s0 + sw],
                        in_=dps[:, :sw],
                        func=act,
                        bias=bias[fo][:],
                    )
                if not last:
                    for tt in range(4 * s, min(4 * s + 4, T)):
                        tw = min(128, B - tt * 128)
                        hn_nm = hnp.tile([128, 256], TD, tag="hn_nm")
                        for hh in range(FO // 128):
                            trp = tr_ps.tile([128, 128], BF16, tag="tr", space="PSUM")
                            nc.tensor.transpose(
                                out=trp[:tw, :],
                                in_=nxt[hh][:, tt * 128 : tt * 128 + tw],
                                identity=ident[:],
                            )
                            nc.scalar.activation(
                                out=hn_nm[:tw, hh * 128 : (hh + 1) * 128],
                                in_=trp[:tw, :],
                                func=AF.Copy,
                                scale=dinvc_t[:tw, tt : tt + 1],
                            )
                        nc.scalar.dma_start(
                            out=hn_stage[li % 2][tt * 128 : tt * 128 + tw, :],
                            in_=hn_nm[:tw, :],
                        )
            if not last:
                # ONE AllGather per layer (output is core-major, so node ids
                # index the table; groups read the two row-halves as slices).
                # Emitted AFTER all of this layer's gathers on the Pool queue
                # (a collective between dma_gathers wedges the device).
                nc.gpsimd.collective_compute(
                    "AllGather",
                    mybir.AluOpType.bypass,
                    replica_groups=[list(range(N_CORES))],
                    ins=[hn_stage[li % 2][0:B, :]],
                    outs=[hn_sh[:]],
                )
            cur, nxt = nxt, cur

        nc.sync.dma_start(out=out_p[:], in_=outF[:])

    nc.compile()
    split_sync_waits(nc)
    bass.Bass.finalize(nc)
    return nc


# ---------------------------------------------------------------------------
# Entry point
# ---------------------------------------------------------------------------


def make_in_maps(p, W1, b1, W2, b2, W3, b3):
    in_maps = []
    for c in range(N_CORES):
        in_maps.append(
            {
                "tab1a": p.tab1a,
                "tab1b": p.tab1b,
                "xT": p.xT[c],
                "idx": p.idx16[c],
                "dloc": p.dloc[c],
                "iota": p.iota,
                "dinvc": p.dinvc[c],
                "ndinvc": p.ndinvc[c],
                "W1": np.asarray(W1, np.float32).astype(BFNP),
                "W2": np.asarray(W2, np.float32).astype(BFNP),
                "W3": np.asarray(W3, np.float32).astype(BFNP),
                "b1": np.asarray(b1, np.float32).reshape(-1, 1),
                "b2": np.asarray(b2, np.float32).reshape(-1, 1),
                "b3": np.asarray(b3, np.float32).reshape(-1, 1),
            }
        )
    return in_maps


def kernel(x, src, dst, W1, b1, W2, b2, W3, b3):
    x = np.asarray(x, np.float32)
    src = np.asarray(src, np.int32)
    dst = np.asarray(dst, np.int32)
    p = build_plan(x, src, dst, x.shape[0], h1=H1_CFG)
    nc = build_nc(p)
    in_maps = make_in_maps(p, W1, b1, W2, b2, W3, b3)
    res = run_bass_kernel_spmd(nc, in_maps, list(range(N_CORES))).results
    out = np.empty((x.shape[0], W3.shape[1]), np.float32)
    B = p.B
    for c in range(N_CORES):
        out[c * B : (c + 1) * B, :] = res[c]["outT"].T
    return out
